# revision 9
# baseline (speedup 1.0000x reference)
"""Bidirectional cross-attention kernel for 8 Trainium2 NeuronCores.

Sharding: 16 (batch, head) units across 8 cores -> core c handles
batch b = c//4 and heads (2*(c%4), 2*(c%4)+1).  Each core computes, for its
two heads:
    E   = exp(scale * qk @ m_qk^T)           (unnormalized, shared both ways)
    M1T = [v | 1]^T @ E                       -> m-side out^T + colsum row
    O1T = [m_v | 1]^T @ E^T                   -> x-side out^T + rowsum row
    px  = sum_h (O1T_h / rowsum)^T @ Wof_h    (Wof = Wo @ Wf folded on host)
    pm  = sum_h (M1T_h / colsum)^T @ mWof_h
Host sums the 4 per-batch partials and adds the folded biases.
E^T is produced by a DRAM round-trip through the DMA xbar transpose
(SBUF-source xbar transpose is not supported on this hardware).
"""

import numpy as np
import ml_dtypes

import concourse.bass as bass
import concourse.mybir as mybir
import concourse.tile as tile
from concourse import bacc
from concourse.bass_utils import run_bass_kernel_spmd
from concourse.masks import make_identity

F32 = mybir.dt.float32
BF16 = mybir.dt.bfloat16
EXP = mybir.ActivationFunctionType.Exp

N = 2048          # sequence length (i and j)
DM = 256          # model dim
DH = 64           # head dim
NT = N // 128     # 16 row tiles
SCALE = DH ** -0.5

_cache = {}


def _build():
    nc = bacc.Bacc("TRN2", target_bir_lowering=False, debug=False, num_devices=8)

    xl = nc.dram_tensor("xl", [N, DM], F32, kind="ExternalInput")
    ml = nc.dram_tensor("ml", [N, DM], F32, kind="ExternalInput")
    wqk = nc.dram_tensor("wqk", [DM, 128], F32, kind="ExternalInput")
    mwqk = nc.dram_tensor("mwqk", [DM, 128], F32, kind="ExternalInput")
    wv = nc.dram_tensor("wv", [DM, 128], F32, kind="ExternalInput")
    mwv = nc.dram_tensor("mwv", [DM, 128], F32, kind="ExternalInput")
    wof = nc.dram_tensor("wof", [128, DM], BF16, kind="ExternalInput")
    mwof = nc.dram_tensor("mwof", [128, DM], BF16, kind="ExternalInput")
    px = nc.dram_tensor("px", [N, DM], F32, kind="ExternalOutput")
    pm = nc.dram_tensor("pm", [N, DM], F32, kind="ExternalOutput")

    with tile.TileContext(nc) as tc:
        _emit(tc, xl, ml, wqk, mwqk, wv, mwv, wof, mwof, px, pm)
    nc.compile()
    return nc


def _emit(tc, xl, ml, wqk, mwqk, wv, mwv, wof, mwof, px, pm):
    nc = tc.nc
    import contextlib
    ctx = contextlib.ExitStack()
    with ctx:
        singles = ctx.enter_context(tc.tile_pool(name="singles", bufs=1))
        xin_p = ctx.enter_context(tc.tile_pool(name="xin", bufs=3))
        e_p = ctx.enter_context(tc.tile_pool(name="et", bufs=3))
        et_p = ctx.enter_context(tc.tile_pool(name="ett", bufs=16))
        big_p = ctx.enter_context(tc.tile_pool(name="big", bufs=2))
        bc_p = ctx.enter_context(tc.tile_pool(name="bc", bufs=1))
        out_p = ctx.enter_context(tc.tile_pool(name="outp", bufs=2))
        psim_p = ctx.enter_context(tc.tile_pool(name="psim", bufs=2, space="PSUM"))
        pacc_p = ctx.enter_context(tc.tile_pool(name="pacc", bufs=4, space="PSUM"))
        dram_p = ctx.enter_context(tc.tile_pool(name="dram", bufs=2, space="DRAM"))

        ident = singles.tile([128, 128], F32)
        make_identity(nc, ident)

        # ---- load weights ----
        wqk_sb = singles.tile([128, 2, 128], F32)
        mwqk_sb = singles.tile([128, 2, 128], F32)
        wv_sb = singles.tile([128, 2, 128], F32)
        mwv_sb = singles.tile([128, 2, 128], F32)
        for t, d in ((wqk_sb, wqk), (mwqk_sb, mwqk), (wv_sb, wv), (mwv_sb, mwv)):
            nc.sync.dma_start(out=t[:], in_=d.rearrange("(k p) n -> p k n", p=128))
        wof_sb = singles.tile([64, 2, DM], BF16)
        mwof_sb = singles.tile([64, 2, DM], BF16)
        nc.sync.dma_start(out=wof_sb[:], in_=wof.rearrange("(h d) n -> d h n", d=64))
        nc.sync.dma_start(out=mwof_sb[:], in_=mwof.rearrange("(h d) n -> d h n", d=64))

        # ---- transpose x, m into [dm, n] layout ----
        xT = singles.tile([128, 2, N], F32)   # [dm%128, dm//128, n]
        mT = singles.tile([128, 2, N], F32)
        for src, dst in ((xl, xT), (ml, mT)):
            for t in range(NT):
                xin = xin_p.tile([128, DM], F32, tag="xin")
                nc.sync.dma_start(out=xin[:], in_=src[t * 128:(t + 1) * 128, :])
                for kc in range(2):
                    pt = psim_p.tile([128, 128], F32, tag="psim")
                    nc.tensor.transpose(pt[:], xin[:, kc * 128:(kc + 1) * 128], ident)
                    eng = nc.vector.tensor_copy if (t + kc) % 2 else nc.scalar.copy
                    eng(dst[:, kc, t * 128:(t + 1) * 128], pt[:])

        # ---- projections ----
        qkT = singles.tile([128, N], F32)     # rows 0:64 head0, 64:128 head1
        m_qkT = singles.tile([128, N], F32)
        for w_sb, src, dst in ((wqk_sb, xT, qkT), (mwqk_sb, mT, m_qkT)):
            for nt in range(4):
                pq = pacc_p.tile([128, 512], F32, tag="pacc")
                for kc in range(2):
                    nc.tensor.matmul(pq[:], w_sb[:, kc, :],
                                     src[:, kc, nt * 512:(nt + 1) * 512],
                                     start=(kc == 0), stop=(kc == 1))
                eng = nc.vector.tensor_copy if nt % 2 else nc.scalar.copy
                eng(dst[:, nt * 512:(nt + 1) * 512], pq[:])

        # v / m_v in natural layout with ones column: [128, t, head, 65]
        v_sb = singles.tile([128, NT, 2, 65], BF16)
        mv_sb = singles.tile([128, NT, 2, 65], BF16)
        for w_sb, src, dst in ((wv_sb, xT, v_sb), (mwv_sb, mT, mv_sb)):
            for t in range(NT):
                pv = pacc_p.tile([128, 128], F32, tag="pacc")
                for kc in range(2):
                    nc.tensor.matmul(pv[:], src[:, kc, t * 128:(t + 1) * 128],
                                     w_sb[:, kc, :], start=(kc == 0), stop=(kc == 1))
                eng = nc.vector.tensor_copy if t % 2 else nc.scalar.copy
                eng(dst[:, t, :, 0:64], pv.rearrange("p (h d) -> p h d", h=2))
            nc.vector.memset(dst[:, :, :, 64:65], 1.0)

        Edram = dram_p.tile([N, N], BF16)
        outT_b = singles.tile([64, 2, N], BF16)    # x-side normalized out^T
        m_outT_b = singles.tile([64, 2, N], BF16)  # m-side normalized out^T

        for h in range(2):
            # ---- phase A: sim -> exp -> E, M1T accumulation, E -> DRAM ----
            pM1T = []
            for _jc in range(4):
                pt_m1 = pacc_p.tile([128, 512], F32, tag="pacc")
                pM1T.append(pt_m1)
            for ic in range(NT):
                Et = e_p.tile([128, N], BF16, tag="et")
                for half in range(2):
                    ps = psim_p.tile([128, 1024], F32, tag="psim")
                    for q in range(2):
                        jn = half * 2 + q
                        nc.tensor.matmul(
                            ps[:, q * 512:(q + 1) * 512],
                            qkT[h * 64:(h + 1) * 64, ic * 128:(ic + 1) * 128],
                            m_qkT[h * 64:(h + 1) * 64, jn * 512:(jn + 1) * 512],
                            start=True, stop=True)
                    nc.scalar.activation(Et[:, half * 1024:(half + 1) * 1024],
                                         ps[:], EXP, scale=SCALE)
                nc.sync.dma_start(out=Edram[ic * 128:(ic + 1) * 128, :], in_=Et[:])
                for jc in range(4):
                    nc.tensor.matmul(pM1T[jc][0:65, :], v_sb[:, ic, h, :],
                                     Et[:, jc * 512:(jc + 1) * 512],
                                     start=(ic == 0), stop=(ic == NT - 1))

            # ---- m-side normalize ----
            M1Tf = big_p.tile([65, N], F32, tag="acc_f32")
            for jc in range(4):
                eng = nc.vector.tensor_copy if jc % 2 else nc.scalar.copy
                eng(M1Tf[:, jc * 512:(jc + 1) * 512], pM1T[jc][0:65, :])
            crec = xin_p.tile([1, N], F32, tag="rec")
            nc.vector.reciprocal(crec[:], M1Tf[64:65, :])
            crec_d = dram_p.tile([1, N], F32, tag="rec_d")
            nc.sync.dma_start(out=crec_d[:], in_=crec[:])
            cb = bc_p.tile([64, N], F32, tag="bcast")
            nc.gpsimd.dma_start(
                out=cb[:],
                in_=bass.AP(tensor=crec_d.tensor, offset=crec_d.offset,
                            ap=[[0, 64], [1, N]]))
            nc.vector.tensor_mul(m_outT_b[:, h, :], M1Tf[0:64, :], cb[:])

            # ---- E^T via DRAM xbar transpose, then O1T ----
            ETs = []
            for jt in range(NT):
                et = et_p.tile([128, N], BF16, tag="ett")
                nc.sync.dma_start_transpose(
                    out=et[:], in_=Edram[:, jt * 128:(jt + 1) * 128])
                ETs.append(et)
            O1Tf = big_p.tile([65, N], F32, tag="acc_f32")
            for iN in range(4):
                pO = pacc_p.tile([128, 512], F32, tag="pacc")
                for jt in range(NT):
                    nc.tensor.matmul(pO[0:65, :], mv_sb[:, jt, h, :],
                                     ETs[jt][:, iN * 512:(iN + 1) * 512],
                                     start=(jt == 0), stop=(jt == NT - 1))
                eng = nc.vector.tensor_copy if iN % 2 else nc.scalar.copy
                eng(O1Tf[:, iN * 512:(iN + 1) * 512], pO[0:65, :])
            rrec = xin_p.tile([1, N], F32, tag="rec")
            nc.vector.reciprocal(rrec[:], O1Tf[64:65, :])
            rrec_d = dram_p.tile([1, N], F32, tag="rec_d")
            nc.sync.dma_start(out=rrec_d[:], in_=rrec[:])
            rb = bc_p.tile([64, N], F32, tag="bcast")
            nc.gpsimd.dma_start(
                out=rb[:],
                in_=bass.AP(tensor=rrec_d.tensor, offset=rrec_d.offset,
                            ap=[[0, 64], [1, N]]))
            nc.vector.tensor_mul(outT_b[:, h, :], O1Tf[0:64, :], rb[:])

        # ---- output projections: accumulate both heads ----
        for src, w_sb, dst in ((outT_b, wof_sb, px), (m_outT_b, mwof_sb, pm)):
            for t in range(NT):
                pP = psim_p.tile([128, DM], F32, tag="psim")
                nc.tensor.matmul(pP[:], src[:, 0, t * 128:(t + 1) * 128],
                                 w_sb[:, 0, :], start=True, stop=False)
                nc.tensor.matmul(pP[:], src[:, 1, t * 128:(t + 1) * 128],
                                 w_sb[:, 1, :], start=False, stop=True)
                po = out_p.tile([128, DM], F32, tag="outp")
                eng = nc.vector.tensor_copy if t % 2 else nc.scalar.copy
                eng(po[:], pP[:])
                nc.sync.dma_start(out=dst[t * 128:(t + 1) * 128, :], in_=po[:])


def kernel(x, m, Wqk, mWqk, Wv, mWv, Wo, bo, mWo, mbo, Wf, bf):
    x = np.asarray(x, dtype=np.float32)
    m = np.asarray(m, dtype=np.float32)
    Wqk = np.asarray(Wqk, dtype=np.float32)
    mWqk = np.asarray(mWqk, dtype=np.float32)
    Wv = np.asarray(Wv, dtype=np.float32)
    mWv = np.asarray(mWv, dtype=np.float32)
    Wo = np.asarray(Wo, dtype=np.float32)
    mWo = np.asarray(mWo, dtype=np.float32)
    Wf = np.asarray(Wf, dtype=np.float32)
    bo = np.asarray(bo, dtype=np.float32)
    mbo = np.asarray(mbo, dtype=np.float32)
    bf = np.asarray(bf, dtype=np.float32)

    if "nc" not in _cache:
        _cache["nc"] = _build()
    nc = _cache["nc"]

    wof = (Wo @ Wf).astype(ml_dtypes.bfloat16)    # [512, 256]
    mwof = (mWo @ Wf).astype(ml_dtypes.bfloat16)
    bias_x = bo @ Wf + bf
    bias_m = mbo @ Wf + bf

    in_maps = []
    for c in range(8):
        b, hp = c // 4, c % 4
        cs = slice(hp * 128, (hp + 1) * 128)
        in_maps.append({
            "xl": x[b], "ml": m[b],
            "wqk": np.ascontiguousarray(Wqk[:, cs]),
            "mwqk": np.ascontiguousarray(mWqk[:, cs]),
            "wv": np.ascontiguousarray(Wv[:, cs]),
            "mwv": np.ascontiguousarray(mWv[:, cs]),
            "wof": np.ascontiguousarray(wof[cs, :]),
            "mwof": np.ascontiguousarray(mwof[cs, :]),
        })

    res = run_bass_kernel_spmd(nc, in_maps, list(range(8)))

    out = np.empty((2, 2 * N, DM), dtype=np.float32)
    for b in range(2):
        cores = range(b * 4, b * 4 + 4)
        out[b, :N] = sum(res.results[c]["px"] for c in cores) + bias_x
        out[b, N:] = sum(res.results[c]["pm"] for c in cores) + bias_m
    return out


# revision 19
# speedup vs baseline: 224.0598x; 224.0598x over previous
"""Bidirectional cross-attention kernel for 8 Trainium2 NeuronCores.

Sharding: 16 (batch, head) units across 8 cores -> core c handles
batch b = c//4 and heads (2*(c%4), 2*(c%4)+1).  Each core computes, for its
two heads:
    E   = exp(scale * qk @ m_qk^T)           (unnormalized, shared both ways)
    M1T = [v | 1]^T @ E                       -> m-side out^T + colsum row
    O1T = [m_v | 1]^T @ E^T                   -> x-side out^T + rowsum row
    px  = sum_h (O1T_h / rowsum)^T @ Wof_h    (Wof = Wo @ Wf folded on host)
    pm  = sum_h (M1T_h / colsum)^T @ mWof_h
Host sums the 4 per-batch partials and adds the folded biases.
E^T is produced by a DRAM round-trip through the DMA xbar transpose
(SBUF-source xbar transpose is not supported on this hardware).
"""

import numpy as np
import ml_dtypes

import concourse.bass as bass
import concourse.mybir as mybir
import concourse.tile as tile
from concourse import bacc
from concourse.bass_utils import run_bass_kernel_spmd
from concourse.masks import make_identity

F32 = mybir.dt.float32
BF16 = mybir.dt.bfloat16
EXP = mybir.ActivationFunctionType.Exp

N = 2048          # sequence length (i and j)
DM = 256          # model dim
DH = 64           # head dim
NT = N // 128     # 16 row tiles
SCALE = DH ** -0.5

_cache = {}
CFG = {"et": True, "edram": True, "m1t": True, "sim": True, "proj": True}


def _build():
    nc = bacc.Bacc("TRN2", target_bir_lowering=False, debug=False, num_devices=8)

    xl = nc.dram_tensor("xl", [N, DM], F32, kind="ExternalInput")
    ml = nc.dram_tensor("ml", [N, DM], F32, kind="ExternalInput")
    wqk = nc.dram_tensor("wqk", [DM, 128], F32, kind="ExternalInput")
    mwqk = nc.dram_tensor("mwqk", [DM, 128], F32, kind="ExternalInput")
    wv = nc.dram_tensor("wv", [DM, 128], F32, kind="ExternalInput")
    mwv = nc.dram_tensor("mwv", [DM, 128], F32, kind="ExternalInput")
    wof = nc.dram_tensor("wof", [128, DM], BF16, kind="ExternalInput")
    mwof = nc.dram_tensor("mwof", [128, DM], BF16, kind="ExternalInput")
    px = nc.dram_tensor("px", [N, DM], F32, kind="ExternalOutput")
    pm = nc.dram_tensor("pm", [N, DM], F32, kind="ExternalOutput")

    with tile.TileContext(nc) as tc:
        _emit(tc, xl, ml, wqk, mwqk, wv, mwv, wof, mwof, px, pm)
    nc.compile()
    return nc


def _emit(tc, xl, ml, wqk, mwqk, wv, mwv, wof, mwof, px, pm):
    nc = tc.nc
    import contextlib
    ctx = contextlib.ExitStack()
    with ctx:
        singles = ctx.enter_context(tc.tile_pool(name="singles", bufs=1))
        xin_p = ctx.enter_context(tc.tile_pool(name="xin", bufs=3))
        e_p = ctx.enter_context(tc.tile_pool(name="et", bufs=3))
        et_p = ctx.enter_context(tc.tile_pool(name="ett", bufs=16))
        big_p = ctx.enter_context(tc.tile_pool(name="big", bufs=2))
        bc_p = ctx.enter_context(tc.tile_pool(name="bc", bufs=1))
        out_p = ctx.enter_context(tc.tile_pool(name="outp", bufs=2))
        psim_p = ctx.enter_context(tc.tile_pool(name="psim", bufs=2, space="PSUM"))
        pacc_p = ctx.enter_context(tc.tile_pool(name="pacc", bufs=4, space="PSUM"))
        dram_p = ctx.enter_context(tc.tile_pool(name="dram", bufs=2, space="DRAM"))

        ident = singles.tile([128, 128], F32)
        make_identity(nc, ident)

        # ---- load weights ----
        wqk_sb = singles.tile([128, 2, 128], F32)
        mwqk_sb = singles.tile([128, 2, 128], F32)
        wv_sb = singles.tile([128, 2, 128], F32)
        mwv_sb = singles.tile([128, 2, 128], F32)
        for t, d in ((wqk_sb, wqk), (mwqk_sb, mwqk), (wv_sb, wv), (mwv_sb, mwv)):
            nc.sync.dma_start(out=t[:], in_=d.rearrange("(k p) n -> p k n", p=128))
        wof_sb = singles.tile([64, 2, DM], BF16)
        mwof_sb = singles.tile([64, 2, DM], BF16)
        nc.sync.dma_start(out=wof_sb[:], in_=wof.rearrange("(h d) n -> d h n", d=64))
        nc.sync.dma_start(out=mwof_sb[:], in_=mwof.rearrange("(h d) n -> d h n", d=64))

        # ---- transpose x, m into [dm, n] layout ----
        xT = singles.tile([128, 2, N], F32)   # [dm%128, dm//128, n]
        mT = singles.tile([128, 2, N], F32)
        for src, dst in ((xl, xT), (ml, mT)):
            for t in range(NT):
                xin = xin_p.tile([128, DM], F32, tag="xin")
                nc.sync.dma_start(out=xin[:], in_=src[t * 128:(t + 1) * 128, :])
                for kc in range(2):
                    pt = psim_p.tile([128, 128], F32, tag="psim")
                    nc.tensor.transpose(pt[:], xin[:, kc * 128:(kc + 1) * 128], ident)
                    eng = nc.vector.tensor_copy
                    eng(dst[:, kc, t * 128:(t + 1) * 128], pt[:])

        # ---- projections ----
        qkT = singles.tile([128, N], F32)     # rows 0:64 head0, 64:128 head1
        m_qkT = singles.tile([128, N], F32)
        for w_sb, src, dst in ((wqk_sb, xT, qkT), (mwqk_sb, mT, m_qkT)):
            for nt in range(4):
                pq = pacc_p.tile([128, 512], F32, tag="pacc")
                for kc in range(2):
                    nc.tensor.matmul(pq[:], w_sb[:, kc, :],
                                     src[:, kc, nt * 512:(nt + 1) * 512],
                                     start=(kc == 0), stop=(kc == 1))
                eng = nc.vector.tensor_copy
                eng(dst[:, nt * 512:(nt + 1) * 512], pq[:])

        # v / m_v in natural layout with ones column: [128, t, head, 65]
        v_sb = singles.tile([128, NT, 2, 65], BF16)
        mv_sb = singles.tile([128, NT, 2, 65], BF16)
        for w_sb, src, dst in ((wv_sb, xT, v_sb), (mwv_sb, mT, mv_sb)):
            for t in range(NT):
                pv = pacc_p.tile([128, 128], F32, tag="pacc")
                for kc in range(2):
                    nc.tensor.matmul(pv[:], src[:, kc, t * 128:(t + 1) * 128],
                                     w_sb[:, kc, :], start=(kc == 0), stop=(kc == 1))
                eng = nc.vector.tensor_copy
                eng(dst[:, t, :, 0:64], pv.rearrange("p (h d) -> p h d", h=2))
            nc.vector.memset(dst[:, :, :, 64:65], 1.0)

        outT_b = singles.tile([64, 2, N], BF16)    # x-side normalized out^T
        m_outT_b = singles.tile([64, 2, N], BF16)  # m-side normalized out^T

        Edrams = [nc.dram_tensor("edram0", [N, N], BF16).ap(),
                  nc.dram_tensor("edram1", [N, N], BF16).ap()]
        for h in range(2):
            Edram = Edrams[h]
            # ---- phase A: sim -> exp -> E, M1T accumulation, E -> DRAM ----
            pM1T = []
            for _jc in range(4):
                pt_m1 = pacc_p.tile([128, 512], F32, tag="pacc")
                pM1T.append(pt_m1)
            for ic in range(NT):
                Et = e_p.tile([128, N], BF16, tag="et")
                for half in range(2):
                    ps = psim_p.tile([128, 1024], F32, tag="psim")
                    for q in range(2 if CFG["sim"] else 0):
                        jn = half * 2 + q
                        nc.tensor.matmul(
                            ps[:, q * 512:(q + 1) * 512],
                            qkT[h * 64:(h + 1) * 64, ic * 128:(ic + 1) * 128],
                            m_qkT[h * 64:(h + 1) * 64, jn * 512:(jn + 1) * 512],
                            start=True, stop=True)
                    nc.scalar.activation(Et[:, half * 1024:(half + 1) * 1024],
                                         ps[:], EXP, scale=SCALE)
                if CFG["edram"]:
                    nc.sync.dma_start(out=Edram[ic * 128:(ic + 1) * 128, :], in_=Et[:])
                for jc in range(4 if CFG["m1t"] else 0):
                    nc.tensor.matmul(pM1T[jc][0:65, :], v_sb[:, ic, h, :],
                                     Et[:, jc * 512:(jc + 1) * 512],
                                     start=(ic == 0), stop=(ic == NT - 1))

            # ---- m-side normalize ----
            M1Tf = big_p.tile([65, N], F32, tag="acc_f32")
            for jc in range(4):
                eng = nc.vector.tensor_copy
                eng(M1Tf[:, jc * 512:(jc + 1) * 512], pM1T[jc][0:65, :])
            crec = xin_p.tile([1, N], F32, tag="rec")
            nc.vector.reciprocal(crec[:], M1Tf[64:65, :])
            crec_d = dram_p.tile([1, N], F32, tag="rec_d")
            nc.sync.dma_start(out=crec_d[:], in_=crec[:])
            cb = bc_p.tile([64, N], F32, tag="bcast")
            nc.gpsimd.dma_start(
                out=cb[:],
                in_=bass.AP(tensor=crec_d.tensor, offset=crec_d.offset,
                            ap=[[0, 64], [1, N]]))
            nc.vector.tensor_mul(m_outT_b[:, h, :], M1Tf[0:64, :], cb[:])

        for h in range(2):
            Edram = Edrams[h]
            # ---- x-side via DRAM xbar transpose of E, then O1T ----
            O1Tf = big_p.tile([65, N], F32, tag="acc_f32")
            if CFG["et"]:
                ETs = []
                for jt in range(NT):
                    et = et_p.tile([128, N], BF16, tag="ett")
                    nc.sync.dma_start_transpose(
                        out=et[:], in_=Edram[:, jt * 128:(jt + 1) * 128])
                    ETs.append(et)
                for iN in range(4):
                    pO = pacc_p.tile([128, 512], F32, tag="pacc")
                    for jt in range(NT):
                        nc.tensor.matmul(pO[0:65, :], mv_sb[:, jt, h, :],
                                         ETs[jt][:, iN * 512:(iN + 1) * 512],
                                         start=(jt == 0), stop=(jt == NT - 1))
                    eng = nc.vector.tensor_copy
                    eng(O1Tf[:, iN * 512:(iN + 1) * 512], pO[0:65, :])
            elif CFG["et"]:
                pO1p = []
                for _iN in range(4):
                    pt_o1 = pacc_p.tile([128, 512], F32, tag="pacc")
                    pO1p.append(pt_o1)
                for jc in range(NT):
                    Ep = e_p.tile([128, N], BF16, tag="et")
                    for half in range(2):
                        ps = psim_p.tile([128, 1024], F32, tag="psim")
                        for q in range(2):
                            iN = half * 2 + q
                            nc.tensor.matmul(
                                ps[:, q * 512:(q + 1) * 512],
                                m_qkT[h * 64:(h + 1) * 64, jc * 128:(jc + 1) * 128],
                                qkT[h * 64:(h + 1) * 64, iN * 512:(iN + 1) * 512],
                                start=True, stop=True)
                        nc.scalar.activation(Ep[:, half * 1024:(half + 1) * 1024],
                                             ps[:], EXP, scale=SCALE)
                    for iN in range(4):
                        nc.tensor.matmul(pO1p[iN][0:65, :], mv_sb[:, jc, h, :],
                                         Ep[:, iN * 512:(iN + 1) * 512],
                                         start=(jc == 0), stop=(jc == NT - 1))
                for iN in range(4):
                    eng = nc.vector.tensor_copy
                    eng(O1Tf[:, iN * 512:(iN + 1) * 512], pO1p[iN][0:65, :])
            else:
                nc.vector.memset(O1Tf[:], 1.0)
            rrec = xin_p.tile([1, N], F32, tag="rec")
            nc.vector.reciprocal(rrec[:], O1Tf[64:65, :])
            rrec_d = dram_p.tile([1, N], F32, tag="rec_d")
            nc.sync.dma_start(out=rrec_d[:], in_=rrec[:])
            rb = bc_p.tile([64, N], F32, tag="bcast")
            nc.gpsimd.dma_start(
                out=rb[:],
                in_=bass.AP(tensor=rrec_d.tensor, offset=rrec_d.offset,
                            ap=[[0, 64], [1, N]]))
            nc.vector.tensor_mul(outT_b[:, h, :], O1Tf[0:64, :], rb[:])

        # ---- output projections: accumulate both heads ----
        for src, w_sb, dst in ((outT_b, wof_sb, px), (m_outT_b, mwof_sb, pm)):
            for t in range(NT):
                pP = psim_p.tile([128, DM], F32, tag="psim")
                nc.tensor.matmul(pP[:], src[:, 0, t * 128:(t + 1) * 128],
                                 w_sb[:, 0, :], start=True, stop=False)
                nc.tensor.matmul(pP[:], src[:, 1, t * 128:(t + 1) * 128],
                                 w_sb[:, 1, :], start=False, stop=True)
                po = out_p.tile([128, DM], F32, tag="outp")
                eng = nc.vector.tensor_copy
                eng(po[:], pP[:])
                nc.sync.dma_start(out=dst[t * 128:(t + 1) * 128, :], in_=po[:])


def kernel(x, m, Wqk, mWqk, Wv, mWv, Wo, bo, mWo, mbo, Wf, bf):
    x = np.asarray(x, dtype=np.float32)
    m = np.asarray(m, dtype=np.float32)
    Wqk = np.asarray(Wqk, dtype=np.float32)
    mWqk = np.asarray(mWqk, dtype=np.float32)
    Wv = np.asarray(Wv, dtype=np.float32)
    mWv = np.asarray(mWv, dtype=np.float32)
    Wo = np.asarray(Wo, dtype=np.float32)
    mWo = np.asarray(mWo, dtype=np.float32)
    Wf = np.asarray(Wf, dtype=np.float32)
    bo = np.asarray(bo, dtype=np.float32)
    mbo = np.asarray(mbo, dtype=np.float32)
    bf = np.asarray(bf, dtype=np.float32)

    if "nc" not in _cache:
        _cache["nc"] = _build()
    nc = _cache["nc"]

    wof = (Wo @ Wf).astype(ml_dtypes.bfloat16)    # [512, 256]
    mwof = (mWo @ Wf).astype(ml_dtypes.bfloat16)
    bias_x = bo @ Wf + bf
    bias_m = mbo @ Wf + bf

    in_maps = []
    for c in range(8):
        b, hp = c // 4, c % 4
        cs = slice(hp * 128, (hp + 1) * 128)
        in_maps.append({
            "xl": x[b], "ml": m[b],
            "wqk": np.ascontiguousarray(Wqk[:, cs]),
            "mwqk": np.ascontiguousarray(mWqk[:, cs]),
            "wv": np.ascontiguousarray(Wv[:, cs]),
            "mwv": np.ascontiguousarray(mWv[:, cs]),
            "wof": np.ascontiguousarray(wof[cs, :]),
            "mwof": np.ascontiguousarray(mwof[cs, :]),
        })

    res = run_bass_kernel_spmd(nc, in_maps, list(range(8)))

    out = np.empty((2, 2 * N, DM), dtype=np.float32)
    for b in range(2):
        cores = range(b * 4, b * 4 + 4)
        out[b, :N] = sum(res.results[c]["px"] for c in cores) + bias_x
        out[b, N:] = sum(res.results[c]["pm"] for c in cores) + bias_m
    return out


# revision 24
# speedup vs baseline: 229.7113x; 1.0252x over previous
"""Bidirectional cross-attention kernel for 8 Trainium2 NeuronCores.

Sharding: 16 (batch, head) units across 8 cores -> core c handles
batch b = c//4 and heads (2*(c%4), 2*(c%4)+1).  Each core computes, for its
two heads:
    E   = exp(scale * qk @ m_qk^T)           (unnormalized, shared both ways)
    M1T = [v | 1]^T @ E                       -> m-side out^T + colsum row
    O1T = [m_v | 1]^T @ E^T                   -> x-side out^T + rowsum row
    px  = sum_h (O1T_h / rowsum)^T @ Wof_h    (Wof = Wo @ Wf folded on host)
    pm  = sum_h (M1T_h / colsum)^T @ mWof_h
Host sums the 4 per-batch partials and adds the folded biases.
E^T is produced by a DRAM round-trip through the DMA xbar transpose
(SBUF-source xbar transpose is not supported on this hardware).
"""

import numpy as np
import ml_dtypes

import concourse.bass as bass
import concourse.mybir as mybir
import concourse.tile as tile
from concourse import bacc
from concourse.bass_utils import run_bass_kernel_spmd
from concourse.masks import make_identity

F32 = mybir.dt.float32
BF16 = mybir.dt.bfloat16
EXP = mybir.ActivationFunctionType.Exp

N = 2048          # sequence length (i and j)
DM = 256          # model dim
DH = 64           # head dim
NT = N // 128     # 16 row tiles
SCALE = DH ** -0.5

_cache = {}
CFG = {"et": True, "edram": True, "m1t": True, "sim": True, "proj": True}


def _build():
    nc = bacc.Bacc("TRN2", target_bir_lowering=False, debug=False, num_devices=8)

    xl = nc.dram_tensor("xl", [N, DM], F32, kind="ExternalInput")
    ml = nc.dram_tensor("ml", [N, DM], F32, kind="ExternalInput")
    wqk = nc.dram_tensor("wqk", [DM, 128], F32, kind="ExternalInput")
    mwqk = nc.dram_tensor("mwqk", [DM, 128], F32, kind="ExternalInput")
    wv = nc.dram_tensor("wv", [DM, 128], F32, kind="ExternalInput")
    mwv = nc.dram_tensor("mwv", [DM, 128], F32, kind="ExternalInput")
    wof = nc.dram_tensor("wof", [128, DM], BF16, kind="ExternalInput")
    mwof = nc.dram_tensor("mwof", [128, DM], BF16, kind="ExternalInput")
    px = nc.dram_tensor("px", [N, DM], F32, kind="ExternalOutput")
    pm = nc.dram_tensor("pm", [N, DM], F32, kind="ExternalOutput")

    with tile.TileContext(nc) as tc:
        _emit(tc, xl, ml, wqk, mwqk, wv, mwv, wof, mwof, px, pm)
    nc.compile()
    return nc


def _emit(tc, xl, ml, wqk, mwqk, wv, mwv, wof, mwof, px, pm):
    nc = tc.nc
    import contextlib
    ctx = contextlib.ExitStack()
    with ctx:
        singles = ctx.enter_context(tc.tile_pool(name="singles", bufs=1))
        xin_p = ctx.enter_context(tc.tile_pool(name="xin", bufs=3))
        rec_p = ctx.enter_context(tc.tile_pool(name="rec", bufs=2))
        e_p = ctx.enter_context(tc.tile_pool(name="et", bufs=4))
        et_p = ctx.enter_context(tc.tile_pool(name="ett", bufs=16))
        big_p = ctx.enter_context(tc.tile_pool(name="big", bufs=2))
        bc_p = ctx.enter_context(tc.tile_pool(name="bc", bufs=1))
        out_p = ctx.enter_context(tc.tile_pool(name="outp", bufs=2))
        psim_p = ctx.enter_context(tc.tile_pool(name="psim", bufs=2, space="PSUM"))
        pacc_p = ctx.enter_context(tc.tile_pool(name="pacc", bufs=4, space="PSUM"))
        dram_p = ctx.enter_context(tc.tile_pool(name="dram", bufs=2, space="DRAM"))

        ident = singles.tile([128, 128], F32)
        make_identity(nc, ident)

        # ---- load weights ----
        wqk_sb = singles.tile([128, 2, 128], F32)
        mwqk_sb = singles.tile([128, 2, 128], F32)
        wv_sb = singles.tile([128, 2, 128], F32)
        mwv_sb = singles.tile([128, 2, 128], F32)
        for t, d in ((wqk_sb, wqk), (mwqk_sb, mwqk), (wv_sb, wv), (mwv_sb, mwv)):
            nc.sync.dma_start(out=t[:], in_=d.rearrange("(k p) n -> p k n", p=128))
        wof_sb = singles.tile([64, 2, DM], BF16)
        mwof_sb = singles.tile([64, 2, DM], BF16)
        nc.sync.dma_start(out=wof_sb[:], in_=wof.rearrange("(h d) n -> d h n", d=64))
        nc.sync.dma_start(out=mwof_sb[:], in_=mwof.rearrange("(h d) n -> d h n", d=64))

        # ---- transpose x, m into [dm, n] layout ----
        xT = singles.tile([128, 2, N], F32)   # [dm%128, dm//128, n]
        mT = singles.tile([128, 2, N], F32)
        for src, dst in ((xl, xT), (ml, mT)):
            for t in range(NT):
                xin = xin_p.tile([128, DM], F32, tag="xin")
                nc.sync.dma_start(out=xin[:], in_=src[t * 128:(t + 1) * 128, :])
                for kc in range(2):
                    pt = psim_p.tile([128, 128], F32, tag="psim")
                    nc.tensor.transpose(pt[:], xin[:, kc * 128:(kc + 1) * 128], ident)
                    eng = nc.vector.tensor_copy
                    eng(dst[:, kc, t * 128:(t + 1) * 128], pt[:])

        # ---- projections ----
        qkT = singles.tile([128, N], F32)     # rows 0:64 head0, 64:128 head1
        m_qkT = singles.tile([128, N], F32)
        for w_sb, src, dst in ((wqk_sb, xT, qkT), (mwqk_sb, mT, m_qkT)):
            for nt in range(4):
                pq = pacc_p.tile([128, 512], F32, tag="pacc")
                for kc in range(2):
                    nc.tensor.matmul(pq[:], w_sb[:, kc, :],
                                     src[:, kc, nt * 512:(nt + 1) * 512],
                                     start=(kc == 0), stop=(kc == 1))
                eng = nc.vector.tensor_copy
                eng(dst[:, nt * 512:(nt + 1) * 512], pq[:])

        # v / m_v in natural layout with ones column: [128, t, head, 65]
        v_sb = singles.tile([128, NT, 2, 65], BF16)
        mv_sb = singles.tile([128, NT, 2, 65], BF16)
        for w_sb, src, dst in ((wv_sb, xT, v_sb), (mwv_sb, mT, mv_sb)):
            for t in range(NT):
                pv = pacc_p.tile([128, 128], F32, tag="pacc")
                for kc in range(2):
                    nc.tensor.matmul(pv[:], src[:, kc, t * 128:(t + 1) * 128],
                                     w_sb[:, kc, :], start=(kc == 0), stop=(kc == 1))
                eng = nc.vector.tensor_copy
                eng(dst[:, t, :, 0:64], pv.rearrange("p (h d) -> p h d", h=2))
            nc.vector.memset(dst[:, :, :, 64:65], 1.0)

        outT_b = singles.tile([64, 2, N], BF16)    # x-side normalized out^T
        m_outT_b = singles.tile([64, 2, N], BF16)  # m-side normalized out^T

        Edrams = [[nc.dram_tensor(f"edram{h}_{hf}", [N // 2, N], BF16).ap()
                   for hf in range(2)] for h in range(2)]
        for h in range(2):
            Edram = Edrams[h]
            # ---- phase A: sim -> exp -> E, M1T accumulation, E -> DRAM ----
            pM1T = []
            for _jc in range(4):
                pt_m1 = pacc_p.tile([128, 512], F32, tag="pacc")
                pM1T.append(pt_m1)
            for ic in range(NT):
                Et = e_p.tile([128, N], BF16, tag="et")
                for half in range(2):
                    ps = psim_p.tile([128, 1024], F32, tag="psim")
                    for q in range(2 if CFG["sim"] else 0):
                        jn = half * 2 + q
                        nc.tensor.matmul(
                            ps[:, q * 512:(q + 1) * 512],
                            qkT[h * 64:(h + 1) * 64, ic * 128:(ic + 1) * 128],
                            m_qkT[h * 64:(h + 1) * 64, jn * 512:(jn + 1) * 512],
                            start=True, stop=True)
                    nc.scalar.activation(Et[:, half * 1024:(half + 1) * 1024],
                                         ps[:], EXP, scale=SCALE)
                if CFG["edram"]:
                    nc.gpsimd.dma_start(
                        out=Edram[ic // 8][(ic % 8) * 128:(ic % 8 + 1) * 128, :],
                        in_=Et[:])
                for jc in range(4 if CFG["m1t"] else 0):
                    nc.tensor.matmul(pM1T[jc][0:65, :], v_sb[:, ic, h, :],
                                     Et[:, jc * 512:(jc + 1) * 512],
                                     start=(ic == 0), stop=(ic == NT - 1))

            # ---- m-side normalize ----
            M1Tf = big_p.tile([65, N], F32, tag="acc_f32")
            for jc in range(4):
                eng = nc.vector.tensor_copy
                eng(M1Tf[:, jc * 512:(jc + 1) * 512], pM1T[jc][0:65, :])
            crec = rec_p.tile([1, N], F32, tag="rec")
            nc.vector.reciprocal(crec[:], M1Tf[64:65, :])
            crec_d = dram_p.tile([1, N], F32, tag="rec_d")
            nc.sync.dma_start(out=crec_d[:], in_=crec[:])
            cb = bc_p.tile([64, N], F32, tag="bcast")
            nc.gpsimd.dma_start(
                out=cb[:],
                in_=bass.AP(tensor=crec_d.tensor, offset=crec_d.offset,
                            ap=[[0, 64], [1, N]]))
            nc.vector.tensor_mul(m_outT_b[:, h, :], M1Tf[0:64, :], cb[:])

            # hoist head-0's E^T xbar reads so they overlap head-1's phase A
            if h == 0 and CFG["et"]:
                ETs0 = []
                for jt in range(NT):
                    et0 = et_p.tile([128, N], BF16, tag="ett")
                    for hf in range(2):
                        nc.sync.dma_start_transpose(
                            out=et0[:, hf * 1024:(hf + 1) * 1024],
                            in_=Edram[hf][:, jt * 128:(jt + 1) * 128])
                    ETs0.append(et0)

        for h in range(2):
            Edram = Edrams[h]
            # ---- x-side via DRAM xbar transpose of E, then O1T ----
            O1Tf = big_p.tile([65, N], F32, tag="acc_f32")
            if CFG["et"]:
                if h == 0:
                    ETs = ETs0
                else:
                    ETs = []
                    for jt in range(NT):
                        et = et_p.tile([128, N], BF16, tag="ett")
                        for hf in range(2):
                            nc.sync.dma_start_transpose(
                                out=et[:, hf * 1024:(hf + 1) * 1024],
                                in_=Edram[hf][:, jt * 128:(jt + 1) * 128])
                        ETs.append(et)
                for iN in range(4):
                    pO = pacc_p.tile([128, 512], F32, tag="pacc")
                    for jt in range(NT):
                        nc.tensor.matmul(pO[0:65, :], mv_sb[:, jt, h, :],
                                         ETs[jt][:, iN * 512:(iN + 1) * 512],
                                         start=(jt == 0), stop=(jt == NT - 1))
                    eng = nc.vector.tensor_copy
                    eng(O1Tf[:, iN * 512:(iN + 1) * 512], pO[0:65, :])
            elif CFG["et"]:
                pO1p = []
                for _iN in range(4):
                    pt_o1 = pacc_p.tile([128, 512], F32, tag="pacc")
                    pO1p.append(pt_o1)
                for jc in range(NT):
                    Ep = e_p.tile([128, N], BF16, tag="et")
                    for half in range(2):
                        ps = psim_p.tile([128, 1024], F32, tag="psim")
                        for q in range(2):
                            iN = half * 2 + q
                            nc.tensor.matmul(
                                ps[:, q * 512:(q + 1) * 512],
                                m_qkT[h * 64:(h + 1) * 64, jc * 128:(jc + 1) * 128],
                                qkT[h * 64:(h + 1) * 64, iN * 512:(iN + 1) * 512],
                                start=True, stop=True)
                        nc.scalar.activation(Ep[:, half * 1024:(half + 1) * 1024],
                                             ps[:], EXP, scale=SCALE)
                    for iN in range(4):
                        nc.tensor.matmul(pO1p[iN][0:65, :], mv_sb[:, jc, h, :],
                                         Ep[:, iN * 512:(iN + 1) * 512],
                                         start=(jc == 0), stop=(jc == NT - 1))
                for iN in range(4):
                    eng = nc.vector.tensor_copy
                    eng(O1Tf[:, iN * 512:(iN + 1) * 512], pO1p[iN][0:65, :])
            else:
                nc.vector.memset(O1Tf[:], 1.0)
            rrec = rec_p.tile([1, N], F32, tag="rec")
            nc.vector.reciprocal(rrec[:], O1Tf[64:65, :])
            rrec_d = dram_p.tile([1, N], F32, tag="rec_d")
            nc.sync.dma_start(out=rrec_d[:], in_=rrec[:])
            rb = bc_p.tile([64, N], F32, tag="bcast")
            nc.gpsimd.dma_start(
                out=rb[:],
                in_=bass.AP(tensor=rrec_d.tensor, offset=rrec_d.offset,
                            ap=[[0, 64], [1, N]]))
            nc.vector.tensor_mul(outT_b[:, h, :], O1Tf[0:64, :], rb[:])

        # ---- output projections: accumulate both heads ----
        for src, w_sb, dst in ((outT_b, wof_sb, px), (m_outT_b, mwof_sb, pm)):
            for t in range(NT):
                pP = psim_p.tile([128, DM], F32, tag="psim")
                nc.tensor.matmul(pP[:], src[:, 0, t * 128:(t + 1) * 128],
                                 w_sb[:, 0, :], start=True, stop=False)
                nc.tensor.matmul(pP[:], src[:, 1, t * 128:(t + 1) * 128],
                                 w_sb[:, 1, :], start=False, stop=True)
                po = out_p.tile([128, DM], F32, tag="outp")
                eng = nc.vector.tensor_copy
                eng(po[:], pP[:])
                nc.sync.dma_start(out=dst[t * 128:(t + 1) * 128, :], in_=po[:])


def kernel(x, m, Wqk, mWqk, Wv, mWv, Wo, bo, mWo, mbo, Wf, bf):
    x = np.asarray(x, dtype=np.float32)
    m = np.asarray(m, dtype=np.float32)
    Wqk = np.asarray(Wqk, dtype=np.float32)
    mWqk = np.asarray(mWqk, dtype=np.float32)
    Wv = np.asarray(Wv, dtype=np.float32)
    mWv = np.asarray(mWv, dtype=np.float32)
    Wo = np.asarray(Wo, dtype=np.float32)
    mWo = np.asarray(mWo, dtype=np.float32)
    Wf = np.asarray(Wf, dtype=np.float32)
    bo = np.asarray(bo, dtype=np.float32)
    mbo = np.asarray(mbo, dtype=np.float32)
    bf = np.asarray(bf, dtype=np.float32)

    if "nc" not in _cache:
        _cache["nc"] = _build()
    nc = _cache["nc"]

    wof = (Wo @ Wf).astype(ml_dtypes.bfloat16)    # [512, 256]
    mwof = (mWo @ Wf).astype(ml_dtypes.bfloat16)
    bias_x = bo @ Wf + bf
    bias_m = mbo @ Wf + bf

    in_maps = []
    for c in range(8):
        b, hp = c // 4, c % 4
        cs = slice(hp * 128, (hp + 1) * 128)
        in_maps.append({
            "xl": x[b], "ml": m[b],
            "wqk": np.ascontiguousarray(Wqk[:, cs]),
            "mwqk": np.ascontiguousarray(mWqk[:, cs]),
            "wv": np.ascontiguousarray(Wv[:, cs]),
            "mwv": np.ascontiguousarray(mWv[:, cs]),
            "wof": np.ascontiguousarray(wof[cs, :]),
            "mwof": np.ascontiguousarray(mwof[cs, :]),
        })

    res = run_bass_kernel_spmd(nc, in_maps, list(range(8)))

    out = np.empty((2, 2 * N, DM), dtype=np.float32)
    for b in range(2):
        cores = range(b * 4, b * 4 + 4)
        out[b, :N] = sum(res.results[c]["px"] for c in cores) + bias_x
        out[b, N:] = sum(res.results[c]["pm"] for c in cores) + bias_m
    return out


# revision 25
# speedup vs baseline: 233.9299x; 1.0184x over previous
"""Bidirectional cross-attention kernel for 8 Trainium2 NeuronCores.

Sharding: 16 (batch, head) units across 8 cores -> core c handles
batch b = c//4 and heads (2*(c%4), 2*(c%4)+1).  Each core computes, for its
two heads:
    E   = exp(scale * qk @ m_qk^T)           (unnormalized, shared both ways)
    M1T = [v | 1]^T @ E                       -> m-side out^T + colsum row
    O1T = [m_v | 1]^T @ E^T                   -> x-side out^T + rowsum row
    px  = sum_h (O1T_h / rowsum)^T @ Wof_h    (Wof = Wo @ Wf folded on host)
    pm  = sum_h (M1T_h / colsum)^T @ mWof_h
Host sums the 4 per-batch partials and adds the folded biases.
E^T is produced by a DRAM round-trip through the DMA xbar transpose
(SBUF-source xbar transpose is not supported on this hardware).
"""

import numpy as np
import ml_dtypes

import concourse.bass as bass
import concourse.mybir as mybir
import concourse.tile as tile
from concourse import bacc
from concourse.bass_utils import run_bass_kernel_spmd
from concourse.masks import make_identity

F32 = mybir.dt.float32
BF16 = mybir.dt.bfloat16
EXP = mybir.ActivationFunctionType.Exp

N = 2048          # sequence length (i and j)
DM = 256          # model dim
DH = 64           # head dim
NT = N // 128     # 16 row tiles
SCALE = DH ** -0.5

_cache = {}
CFG = {"et": True, "edram": True, "m1t": True, "sim": True, "proj": True}


def _build():
    nc = bacc.Bacc("TRN2", target_bir_lowering=False, debug=False, num_devices=8)

    xl = nc.dram_tensor("xl", [N, DM], F32, kind="ExternalInput")
    ml = nc.dram_tensor("ml", [N, DM], F32, kind="ExternalInput")
    wqk = nc.dram_tensor("wqk", [DM, 128], F32, kind="ExternalInput")
    mwqk = nc.dram_tensor("mwqk", [DM, 128], F32, kind="ExternalInput")
    wv = nc.dram_tensor("wv", [DM, 128], F32, kind="ExternalInput")
    mwv = nc.dram_tensor("mwv", [DM, 128], F32, kind="ExternalInput")
    wof = nc.dram_tensor("wof", [128, DM], BF16, kind="ExternalInput")
    mwof = nc.dram_tensor("mwof", [128, DM], BF16, kind="ExternalInput")
    px = nc.dram_tensor("px", [N, DM], F32, kind="ExternalOutput")
    pm = nc.dram_tensor("pm", [N, DM], F32, kind="ExternalOutput")

    with tile.TileContext(nc) as tc:
        _emit(tc, xl, ml, wqk, mwqk, wv, mwv, wof, mwof, px, pm)
    nc.compile()
    return nc


def _emit(tc, xl, ml, wqk, mwqk, wv, mwv, wof, mwof, px, pm):
    nc = tc.nc
    import contextlib
    ctx = contextlib.ExitStack()
    with ctx:
        singles = ctx.enter_context(tc.tile_pool(name="singles", bufs=1))
        xin_p = ctx.enter_context(tc.tile_pool(name="xin", bufs=5))
        rec_p = ctx.enter_context(tc.tile_pool(name="rec", bufs=2))
        e_p = ctx.enter_context(tc.tile_pool(name="et", bufs=4))
        et_p = ctx.enter_context(tc.tile_pool(name="ett", bufs=16))
        big_p = ctx.enter_context(tc.tile_pool(name="big", bufs=2))
        bc_p = ctx.enter_context(tc.tile_pool(name="bc", bufs=1))
        out_p = ctx.enter_context(tc.tile_pool(name="outp", bufs=2))
        psim_p = ctx.enter_context(tc.tile_pool(name="psim", bufs=2, space="PSUM"))
        pacc_p = ctx.enter_context(tc.tile_pool(name="pacc", bufs=4, space="PSUM"))
        dram_p = ctx.enter_context(tc.tile_pool(name="dram", bufs=2, space="DRAM"))

        ident = singles.tile([128, 128], F32)
        make_identity(nc, ident)

        # ---- load weights ----
        wqk_sb = singles.tile([128, 2, 128], F32)
        mwqk_sb = singles.tile([128, 2, 128], F32)
        wv_sb = singles.tile([128, 2, 128], F32)
        mwv_sb = singles.tile([128, 2, 128], F32)
        for t, d in ((wqk_sb, wqk), (mwqk_sb, mwqk), (wv_sb, wv), (mwv_sb, mwv)):
            nc.sync.dma_start(out=t[:], in_=d.rearrange("(k p) n -> p k n", p=128))
        wof_sb = singles.tile([64, 2, DM], BF16)
        mwof_sb = singles.tile([64, 2, DM], BF16)
        nc.sync.dma_start(out=wof_sb[:], in_=wof.rearrange("(h d) n -> d h n", d=64))
        nc.sync.dma_start(out=mwof_sb[:], in_=mwof.rearrange("(h d) n -> d h n", d=64))

        # ---- transpose x, m into [dm, n] layout ----
        xT = singles.tile([128, 2, N], F32)   # [dm%128, dm//128, n]
        mT = singles.tile([128, 2, N], F32)
        for src, dst in ((xl, xT), (ml, mT)):
            for tg in range(NT // 4):
                xins = []
                for q in range(4):
                    xin = xin_p.tile([128, DM], F32, tag="xin")
                    t = tg * 4 + q
                    nc.sync.dma_start(out=xin[:], in_=src[t * 128:(t + 1) * 128, :])
                    xins.append(xin)
                for kc in range(2):
                    pt = psim_p.tile([128, 512], F32, tag="psim")
                    for q in range(4):
                        nc.tensor.transpose(pt[:, q * 128:(q + 1) * 128],
                                            xins[q][:, kc * 128:(kc + 1) * 128], ident)
                    nc.vector.tensor_copy(
                        dst[:, kc, tg * 512:(tg + 1) * 512], pt[:])

        # ---- projections ----
        qkT = singles.tile([128, N], F32)     # rows 0:64 head0, 64:128 head1
        m_qkT = singles.tile([128, N], F32)
        for w_sb, src, dst in ((wqk_sb, xT, qkT), (mwqk_sb, mT, m_qkT)):
            for nt in range(4):
                pq = pacc_p.tile([128, 512], F32, tag="pacc")
                for kc in range(2):
                    nc.tensor.matmul(pq[:], w_sb[:, kc, :],
                                     src[:, kc, nt * 512:(nt + 1) * 512],
                                     start=(kc == 0), stop=(kc == 1))
                eng = nc.vector.tensor_copy
                eng(dst[:, nt * 512:(nt + 1) * 512], pq[:])

        # v / m_v in natural layout with ones column: [128, t, head, 65]
        v_sb = singles.tile([128, NT, 2, 65], BF16)
        mv_sb = singles.tile([128, NT, 2, 65], BF16)
        for w_sb, src, dst in ((wv_sb, xT, v_sb), (mwv_sb, mT, mv_sb)):
            for t in range(NT):
                pv = pacc_p.tile([128, 128], F32, tag="pacc")
                for kc in range(2):
                    nc.tensor.matmul(pv[:], src[:, kc, t * 128:(t + 1) * 128],
                                     w_sb[:, kc, :], start=(kc == 0), stop=(kc == 1))
                eng = nc.vector.tensor_copy
                eng(dst[:, t, :, 0:64], pv.rearrange("p (h d) -> p h d", h=2))
            nc.vector.memset(dst[:, :, :, 64:65], 1.0)

        outT_b = singles.tile([64, 2, N], BF16)    # x-side normalized out^T
        m_outT_b = singles.tile([64, 2, N], BF16)  # m-side normalized out^T

        Edrams = [[nc.dram_tensor(f"edram{h}_{hf}", [N // 2, N], BF16).ap()
                   for hf in range(2)] for h in range(2)]
        for h in range(2):
            Edram = Edrams[h]
            # ---- phase A: sim -> exp -> E, M1T accumulation, E -> DRAM ----
            pM1T = []
            for _jc in range(4):
                pt_m1 = pacc_p.tile([128, 512], F32, tag="pacc")
                pM1T.append(pt_m1)
            for ic in range(NT):
                Et = e_p.tile([128, N], BF16, tag="et")
                for half in range(2):
                    ps = psim_p.tile([128, 1024], F32, tag="psim")
                    for q in range(2 if CFG["sim"] else 0):
                        jn = half * 2 + q
                        nc.tensor.matmul(
                            ps[:, q * 512:(q + 1) * 512],
                            qkT[h * 64:(h + 1) * 64, ic * 128:(ic + 1) * 128],
                            m_qkT[h * 64:(h + 1) * 64, jn * 512:(jn + 1) * 512],
                            start=True, stop=True)
                    nc.scalar.activation(Et[:, half * 1024:(half + 1) * 1024],
                                         ps[:], EXP, scale=SCALE)
                if CFG["edram"]:
                    nc.gpsimd.dma_start(
                        out=Edram[ic // 8][(ic % 8) * 128:(ic % 8 + 1) * 128, :],
                        in_=Et[:])
                for jc in range(4 if CFG["m1t"] else 0):
                    nc.tensor.matmul(pM1T[jc][0:65, :], v_sb[:, ic, h, :],
                                     Et[:, jc * 512:(jc + 1) * 512],
                                     start=(ic == 0), stop=(ic == NT - 1))

            # ---- m-side normalize ----
            M1Tf = big_p.tile([65, N], F32, tag="acc_f32")
            for jc in range(4):
                eng = nc.vector.tensor_copy
                eng(M1Tf[:, jc * 512:(jc + 1) * 512], pM1T[jc][0:65, :])
            crec = rec_p.tile([1, N], F32, tag="rec")
            nc.vector.reciprocal(crec[:], M1Tf[64:65, :])
            crec_d = dram_p.tile([1, N], F32, tag="rec_d")
            nc.sync.dma_start(out=crec_d[:], in_=crec[:])
            cb = bc_p.tile([64, N], F32, tag="bcast")
            nc.gpsimd.dma_start(
                out=cb[:],
                in_=bass.AP(tensor=crec_d.tensor, offset=crec_d.offset,
                            ap=[[0, 64], [1, N]]))
            nc.vector.tensor_mul(m_outT_b[:, h, :], M1Tf[0:64, :], cb[:])

            # hoist head-0's E^T xbar reads so they overlap head-1's phase A
            if h == 0 and CFG["et"]:
                ETs0 = []
                for jt in range(NT):
                    et0 = et_p.tile([128, N], BF16, tag="ett")
                    for hf in range(2):
                        nc.sync.dma_start_transpose(
                            out=et0[:, hf * 1024:(hf + 1) * 1024],
                            in_=Edram[hf][:, jt * 128:(jt + 1) * 128])
                    ETs0.append(et0)

        for h in range(2):
            Edram = Edrams[h]
            # ---- x-side via DRAM xbar transpose of E, then O1T ----
            O1Tf = big_p.tile([65, N], F32, tag="acc_f32")
            if CFG["et"]:
                if h == 0:
                    ETs = ETs0
                else:
                    ETs = []
                    for jt in range(NT):
                        et = et_p.tile([128, N], BF16, tag="ett")
                        for hf in range(2):
                            nc.sync.dma_start_transpose(
                                out=et[:, hf * 1024:(hf + 1) * 1024],
                                in_=Edram[hf][:, jt * 128:(jt + 1) * 128])
                        ETs.append(et)
                for iN in range(4):
                    pO = pacc_p.tile([128, 512], F32, tag="pacc")
                    for jt in range(NT):
                        nc.tensor.matmul(pO[0:65, :], mv_sb[:, jt, h, :],
                                         ETs[jt][:, iN * 512:(iN + 1) * 512],
                                         start=(jt == 0), stop=(jt == NT - 1))
                    eng = nc.vector.tensor_copy
                    eng(O1Tf[:, iN * 512:(iN + 1) * 512], pO[0:65, :])
            elif CFG["et"]:
                pO1p = []
                for _iN in range(4):
                    pt_o1 = pacc_p.tile([128, 512], F32, tag="pacc")
                    pO1p.append(pt_o1)
                for jc in range(NT):
                    Ep = e_p.tile([128, N], BF16, tag="et")
                    for half in range(2):
                        ps = psim_p.tile([128, 1024], F32, tag="psim")
                        for q in range(2):
                            iN = half * 2 + q
                            nc.tensor.matmul(
                                ps[:, q * 512:(q + 1) * 512],
                                m_qkT[h * 64:(h + 1) * 64, jc * 128:(jc + 1) * 128],
                                qkT[h * 64:(h + 1) * 64, iN * 512:(iN + 1) * 512],
                                start=True, stop=True)
                        nc.scalar.activation(Ep[:, half * 1024:(half + 1) * 1024],
                                             ps[:], EXP, scale=SCALE)
                    for iN in range(4):
                        nc.tensor.matmul(pO1p[iN][0:65, :], mv_sb[:, jc, h, :],
                                         Ep[:, iN * 512:(iN + 1) * 512],
                                         start=(jc == 0), stop=(jc == NT - 1))
                for iN in range(4):
                    eng = nc.vector.tensor_copy
                    eng(O1Tf[:, iN * 512:(iN + 1) * 512], pO1p[iN][0:65, :])
            else:
                nc.vector.memset(O1Tf[:], 1.0)
            rrec = rec_p.tile([1, N], F32, tag="rec")
            nc.vector.reciprocal(rrec[:], O1Tf[64:65, :])
            rrec_d = dram_p.tile([1, N], F32, tag="rec_d")
            nc.sync.dma_start(out=rrec_d[:], in_=rrec[:])
            rb = bc_p.tile([64, N], F32, tag="bcast")
            nc.gpsimd.dma_start(
                out=rb[:],
                in_=bass.AP(tensor=rrec_d.tensor, offset=rrec_d.offset,
                            ap=[[0, 64], [1, N]]))
            nc.vector.tensor_mul(outT_b[:, h, :], O1Tf[0:64, :], rb[:])

        # ---- output projections: accumulate both heads ----
        for src, w_sb, dst in ((outT_b, wof_sb, px), (m_outT_b, mwof_sb, pm)):
            for t in range(NT):
                pP = psim_p.tile([128, DM], F32, tag="psim")
                nc.tensor.matmul(pP[:], src[:, 0, t * 128:(t + 1) * 128],
                                 w_sb[:, 0, :], start=True, stop=False)
                nc.tensor.matmul(pP[:], src[:, 1, t * 128:(t + 1) * 128],
                                 w_sb[:, 1, :], start=False, stop=True)
                po = out_p.tile([128, DM], F32, tag="outp")
                eng = nc.vector.tensor_copy
                eng(po[:], pP[:])
                nc.sync.dma_start(out=dst[t * 128:(t + 1) * 128, :], in_=po[:])


def kernel(x, m, Wqk, mWqk, Wv, mWv, Wo, bo, mWo, mbo, Wf, bf):
    x = np.asarray(x, dtype=np.float32)
    m = np.asarray(m, dtype=np.float32)
    Wqk = np.asarray(Wqk, dtype=np.float32)
    mWqk = np.asarray(mWqk, dtype=np.float32)
    Wv = np.asarray(Wv, dtype=np.float32)
    mWv = np.asarray(mWv, dtype=np.float32)
    Wo = np.asarray(Wo, dtype=np.float32)
    mWo = np.asarray(mWo, dtype=np.float32)
    Wf = np.asarray(Wf, dtype=np.float32)
    bo = np.asarray(bo, dtype=np.float32)
    mbo = np.asarray(mbo, dtype=np.float32)
    bf = np.asarray(bf, dtype=np.float32)

    if "nc" not in _cache:
        _cache["nc"] = _build()
    nc = _cache["nc"]

    wof = (Wo @ Wf).astype(ml_dtypes.bfloat16)    # [512, 256]
    mwof = (mWo @ Wf).astype(ml_dtypes.bfloat16)
    bias_x = bo @ Wf + bf
    bias_m = mbo @ Wf + bf

    in_maps = []
    for c in range(8):
        b, hp = c // 4, c % 4
        cs = slice(hp * 128, (hp + 1) * 128)
        in_maps.append({
            "xl": x[b], "ml": m[b],
            "wqk": np.ascontiguousarray(Wqk[:, cs]),
            "mwqk": np.ascontiguousarray(mWqk[:, cs]),
            "wv": np.ascontiguousarray(Wv[:, cs]),
            "mwv": np.ascontiguousarray(mWv[:, cs]),
            "wof": np.ascontiguousarray(wof[cs, :]),
            "mwof": np.ascontiguousarray(mwof[cs, :]),
        })

    res = run_bass_kernel_spmd(nc, in_maps, list(range(8)))

    out = np.empty((2, 2 * N, DM), dtype=np.float32)
    for b in range(2):
        cores = range(b * 4, b * 4 + 4)
        out[b, :N] = sum(res.results[c]["px"] for c in cores) + bias_x
        out[b, N:] = sum(res.results[c]["pm"] for c in cores) + bias_m
    return out
